# revision 12
# baseline (speedup 1.0000x reference)
"""Trainium2 Bass kernel for nn_Attention_20933670601301.

Math (per batch b, P[b] in [n, C], n=512, C=256):
    qk    = Wk^T (Wq P[b, id_b] + bq)            (folded on host)
    scores= P[b] @ qk  (+ const that cancels in softmax)
    attn  = softmax(scores)
    t     = P[b]^T attn        -> out = Wv t + bv (applied on host)

Device work is the O(n)-scaling part only: scores, softmax, t.
P is uploaded once per core as fp16 in C-MAJOR layout pt[cp, b, ch, n]
(c = ch*128 + cp), 8 MiB/core -> ~23.5 us DMA floor at 360 GB/s.

Per core (32 batches):
  scores: PE matmuls with P chunks as the STATIONARY operand
          (lhsT = pt[:, b, ch, k-chunk], rhs = qk column, F=1), accumulated
          over ch into one PSUM tile scores_cols [128, 4, 32].
  softmax (groups of batches): PE-transpose score columns to rows [G, 512],
          DVE max, ACT fused exp(+bias)+sum -> UNNORMALIZED attn fp16 and
          esum (host divides by esum), PE-transpose attn back to columns.
  t:      per batch, broadcast attn across partitions (PE transpose of a
          free-broadcast column -> fp16 PSUM [128, 512]), then the free-dim
          weighted reduction t[c] = sum_n pt[c, n]*attn[n] as either
            S-path: DVE scalar_tensor_tensor (fused mul+accum, PSUM in1)
            V-path: ACT copy attn->SBUF, Pool mul, ACT Identity+accum
          split so DVE/Pool/ACT all stay under the DMA roofline.
Emission is software-pipelined across groups (stage machinery) to avoid
in-order queue head-of-line stalls; later groups are smaller to shorten
the post-stream tail.
"""

import numpy as np

B, N, C = 256, 512, 256
NCORES = 8
BL = B // NCORES      # 32 batches per core
NK = N // 128         # 4 n-chunks of 128
# group sizes for the softmax/t pipeline; smaller tail groups
GROUPS = [8, 8, 8, 4, 4]
# batches per group routed via Pool (V-path); the rest use DVE STT (S-path)
NPOOL = [4, 4, 3, 1, 0]

_CACHE = {}


def _build():
    from contextlib import ExitStack

    import concourse.bass as bass
    import concourse.mybir as mybir
    import concourse.tile as tile
    from concourse import bacc
    from concourse.masks import make_identity

    dt = mybir.dt
    AF = mybir.ActivationFunctionType
    OP = mybir.AluOpType
    f32, f16 = dt.float32, dt.float16

    nc = bacc.Bacc("TRN2", target_bir_lowering=False)
    pt_d = nc.dram_tensor("pt", [128, BL, 2, N], f16, kind="ExternalInput")
    qkt_d = nc.dram_tensor("qkt", [128, 2, BL], f16, kind="ExternalInput")
    tt_d = nc.dram_tensor("tt", [128, 2, BL], f32, kind="ExternalOutput")
    es_d = nc.dram_tensor("es", [BL, 1], f32, kind="ExternalOutput")

    g_base = [sum(GROUPS[:i]) for i in range(len(GROUPS))]

    with tile.TileContext(nc) as tc, ExitStack() as ctx:
        consts = ctx.enter_context(tc.tile_pool(name="consts", bufs=1))
        big = ctx.enter_context(tc.tile_pool(name="big", bufs=1))
        sgrp = ctx.enter_context(tc.tile_pool(name="sgrp", bufs=3))
        scrp = ctx.enter_context(tc.tile_pool(name="scrp", bufs=3))
        vprp = ctx.enter_context(tc.tile_pool(name="vprp", bufs=3))
        psS = ctx.enter_context(tc.tile_pool(name="psS", bufs=1, space="PSUM"))
        psN = ctx.enter_context(tc.tile_pool(name="psN", bufs=2, space="PSUM"))
        psB = ctx.enter_context(tc.tile_pool(name="psB", bufs=4, space="PSUM"))

        ident16 = consts.tile([128, 128], f16)
        make_identity(nc, ident16)
        ident32 = consts.tile([128, 128], f32)
        make_identity(nc, ident32)

        qkt_sb = consts.tile([128, 2, BL], f16)
        nc.sync.dma_start(out=qkt_sb, in_=qkt_d[:, :, :])

        pt_all = big.tile([128, BL, 2, N], f16)
        attnT = consts.tile([128, NK, BL], f16)
        tt_sb = consts.tile([128, 2, BL], f32)

        def load_part(b0, nb):
            nc.sync.dma_start(
                out=pt_all[:, b0 : b0 + nb], in_=pt_d[:, b0 : b0 + nb]
            )

        # scores_cols[:, k, b] = sum_c pt[c, b, :, k*128:(k+1)*128]^T qk[b]
        scores_cols = psS.tile([128, NK, BL], f32)
        # attn-column transpose landing slots (ping-pong per group)
        atT_all = psS.tile([128, 2, NK, 8], f16)

        def scores(b):
            for k in range(NK):
                for ch in range(2):
                    nc.tensor.matmul(
                        out=scores_cols[:, k, b : b + 1],
                        lhsT=pt_all[:, b, ch, k * 128 : (k + 1) * 128],
                        rhs=qkt_sb[:, ch, b : b + 1],
                        start=(ch == 0),
                        stop=(ch == 1),
                    )

        grp_state = {}

        def phase_g1(g):
            # scores PSUM -> SBUF (PE transpose input must be SBUF)
            gb, G = g_base[g], GROUPS[g]
            sc_sb = sgrp.tile([128, NK, 8], f32, tag="sc_sb")
            nc.scalar.copy(
                out=sc_sb[:, :, :G], in_=scores_cols[:, :, gb : gb + G]
            )
            grp_state[g] = sc_sb

        def phase_g2(g):
            # columns -> natural rows [G, 512] in PSUM
            G = GROUPS[g]
            sc_sb = grp_state[g]
            sc_ps = psN.tile([8, NK, 128], f32, tag="scnat")
            for k in range(NK):
                nc.tensor.transpose(
                    out=sc_ps[:G, k, :], in_=sc_sb[:, k, :G], identity=ident32
                )
            grp_state[g] = sc_ps

        def phase_g3(g):
            # row max (safe softmax shift)
            G = GROUPS[g]
            sc_ps = grp_state[g]
            smax = sgrp.tile([8, 1], f32, tag="smax")
            nc.vector.tensor_reduce(
                out=smax[:G],
                in_=sc_ps[:G],
                axis=mybir.AxisListType.XY,
                op=OP.max,
            )
            negmax = sgrp.tile([8, 1], f32, tag="negmax")
            nc.vector.tensor_scalar_mul(negmax[:G], smax[:G], -1.0)
            grp_state[g] = (sc_ps, negmax)

        def phase_g4(g):
            # unnormalized attn = exp(s - max) in fp16 + esum for the host
            gb, G = g_base[g], GROUPS[g]
            sc_ps, negmax = grp_state[g]
            attn_nat = sgrp.tile([8, NK, 128], f16, tag="attn_nat")
            esum = sgrp.tile([8, 1], f32, tag="esum")
            nc.scalar.activation(
                out=attn_nat[:G],
                in_=sc_ps[:G],
                func=AF.Exp,
                bias=negmax[:G, 0:1],
                scale=1.0,
                accum_out=esum[:G],
            )
            # issue on the ACT queue right after the producer so the SP
            # load stream never waits on the softmax chain
            nc.scalar.dma_start(out=es_d[gb : gb + G], in_=esum[:G])
            grp_state[g] = attn_nat

        def phase_g5(g):
            # attn rows -> columns [128, NK, G] (bcast-transpose input)
            gb, G = g_base[g], GROUPS[g]
            attn_nat = grp_state.pop(g)
            atT_ps = atT_all[:, g % 2]
            for k in range(NK):
                nc.tensor.transpose(
                    out=atT_ps[:, k, :G],
                    in_=attn_nat[:G, k, :],
                    identity=ident16[:G, :G],
                )
            nc.scalar.copy(
                out=attnT[:, :, gb : gb + G], in_=atT_ps[:, :, :G]
            )

        def phase_t(g, j):
            # t[:, ch, b] = sum_n pt[:, b, ch, n] * attn[b, n]
            # handles the batch PAIR (j, j+1); one PSUM bank per pair
            gb, G = g_base[g], GROUPS[g]
            ab_ps = psB.tile([128, 2, NK, 128], f16, tag="ab")
            for h in range(2):
                b = gb + j + h
                for k in range(NK):
                    nc.tensor.transpose(
                        out=ab_ps[:, h, k, :],
                        in_=attnT[:, k, b : b + 1].to_broadcast([128, 128]),
                        identity=ident16,
                    )
            for h in range(2):
                b = gb + j + h
                ab_flat = ab_ps[:, h].rearrange("p k f -> p (k f)")
                if j + h >= G - NPOOL[g]:
                    # V-path: Pool muls (needs SBUF attn), ACT accumulates
                    ab_sb = vprp.tile([128, N], f16, tag="ab_sb")
                    nc.scalar.copy(out=ab_sb, in_=ab_flat)
                    prod = vprp.tile([128, 2, N], f16, tag="prod")
                    for ch in range(2):
                        nc.gpsimd.tensor_mul(
                            out=prod[:, ch, :],
                            in0=pt_all[:, b, ch, :],
                            in1=ab_sb,
                        )
                    scr = vprp.tile([128, N], f16, tag="vscr")
                    for ch in range(2):
                        nc.scalar.activation(
                            out=scr,
                            in_=prod[:, ch, :],
                            func=AF.Identity,
                            scale=1.0,
                            accum_out=tt_sb[:, ch, b : b + 1],
                        )
                else:
                    # S-path: fused mul+accum on DVE, attn straight from PSUM
                    scr = scrp.tile([128, N], f16, tag="sscr")
                    for ch in range(2):
                        nc.vector.scalar_tensor_tensor(
                            out=scr,
                            in0=pt_all[:, b, ch, :],
                            scalar=1.0,
                            in1=ab_flat,
                            op0=OP.mult,
                            op1=OP.mult,
                            accum_out=tt_sb[:, ch, b : b + 1],
                        )

        # ---- schedule ----
        # stage list: (fn(g), ready offset in completed batches)
        NG = len(GROUPS)
        g_end = [g_base[i] + GROUPS[i] for i in range(NG)]
        stages = [phase_g1, phase_g2, phase_g3, phase_g4, phase_g5]
        nstage = [0] * (len(stages) + 1)
        tcount = [0] * NG  # per-group emitted t-batches

        def run_stages(done):
            # emit group stage s when the group's batches are done loading
            # and the previous stage for that group has been emitted
            for si, fn in enumerate(stages):
                lim = nstage[si - 1] if si else NG
                while nstage[si] < lim and done >= g_end[nstage[si]] + 2:
                    fn(nstage[si])
                    nstage[si] += 1
            # t-work: emit up to two PAIRS per call so the t-stream keeps
            # pace with the loads (16 pairs over 16 iterations)
            emitted = 0
            for g in range(nstage[len(stages) - 1]):
                while (
                    emitted < 2
                    and tcount[g] < GROUPS[g]
                    and done >= g_end[g] + 4
                ):
                    phase_t(g, tcount[g])
                    tcount[g] += 2
                    emitted += 1

        for b0 in range(0, BL, 2):
            load_part(b0, 2)
            scores(b0)
            scores(b0 + 1)
            run_stages(b0 + 2)

        # drain: finish all stages and t-work in dependency order
        for si, fn in enumerate(stages):
            while nstage[si] < NG:
                fn(nstage[si])
                nstage[si] += 1
        for g in range(NG):
            while tcount[g] < GROUPS[g]:
                phase_t(g, tcount[g])
                tcount[g] += 2

        nc.sync.dma_start(out=tt_d[:, :, :], in_=tt_sb)

    nc.compile()
    return nc


LAST_RESULT = None


def kernel(P, id, Wq, bq, Wk, bk, Wv, bv):
    global LAST_RESULT
    from concourse.bass_utils import run_bass_kernel_spmd

    P = np.asarray(P, dtype=np.float32)
    idv = np.asarray(id).astype(np.int64)
    Wq = np.asarray(Wq, dtype=np.float32)
    Wk = np.asarray(Wk, dtype=np.float32)
    Wv = np.asarray(Wv, dtype=np.float32)
    bq = np.asarray(bq, dtype=np.float32)
    bv = np.asarray(bv, dtype=np.float32)

    if "nc" not in _CACHE:
        _CACHE["nc"] = _build()
    nc = _CACHE["nc"]

    # host-side projection folding: qk = Wk^T (Wq p_sel + bq)
    p_sel = P[np.arange(B), idv]                      # [B, C]
    qk = (p_sel @ Wq.T + bq) @ Wk                     # [B, C]

    in_maps = []
    for c in range(NCORES):
        sl = slice(c * BL, (c + 1) * BL)
        # pt[cp, b, ch, n] = P[b, n, ch*128+cp]
        pc = P[sl].reshape(BL, N, 2, 128).transpose(3, 0, 2, 1)
        qc = qk[sl].reshape(BL, 2, 128).transpose(2, 1, 0)
        in_maps.append(
            {
                "pt": np.ascontiguousarray(pc.astype(np.float16)),
                "qkt": np.ascontiguousarray(qc.astype(np.float16)),
            }
        )

    res = run_bass_kernel_spmd(nc, in_maps, core_ids=list(range(NCORES)))
    LAST_RESULT = res

    outs = []
    for c in range(NCORES):
        tt = res.results[c]["tt"]                     # [128, 2, BL] f32
        es = res.results[c]["es"]                     # [BL, 1] f32
        t_un = tt.transpose(2, 1, 0).reshape(BL, C)   # [BL, C]
        outs.append((t_un / es) @ Wv.T + bv)
    return np.concatenate(outs, axis=0).astype(np.float32)


# revision 14
# speedup vs baseline: 1.0968x; 1.0968x over previous
"""Trainium2 Bass kernel for nn_Attention_20933670601301.

Math (per batch b, P[b] in [n, C], n=512, C=256):
    qk    = Wk^T (Wq P[b, id_b] + bq)            (folded on host)
    scores= P[b] @ qk  (+ const that cancels in softmax)
    attn  = softmax(scores)
    t     = P[b]^T attn        -> out = Wv t + bv (applied on host)

Device work is the O(n)-scaling part only: scores, softmax, t.
P is uploaded once per core as fp16 in C-MAJOR layout pt[cp, b, ch, n]
(c = ch*128 + cp), 8 MiB/core -> ~23.5 us DMA floor at 360 GB/s.

Per core (32 batches):
  scores: PE matmuls with P chunks as the STATIONARY operand
          (lhsT = pt[:, b, ch, k-chunk], rhs = qk column, F=1), accumulated
          over ch into one PSUM tile scores_cols [128, 4, 32].
  softmax (groups of batches): PE-transpose score columns to rows [G, 512],
          DVE max, ACT fused exp(+bias)+sum -> UNNORMALIZED attn fp16 and
          esum (host divides by esum), PE-transpose attn back to columns.
  t:      per batch, broadcast attn across partitions (PE transpose of a
          free-broadcast column -> fp16 PSUM [128, 512]), then the free-dim
          weighted reduction t[c] = sum_n pt[c, n]*attn[n] as either
            S-path: DVE scalar_tensor_tensor (fused mul+accum, PSUM in1)
            V-path: ACT copy attn->SBUF, Pool mul, ACT Identity+accum
          split so DVE/Pool/ACT all stay under the DMA roofline.
Emission is software-pipelined across groups (stage machinery) to avoid
in-order queue head-of-line stalls; later groups are smaller to shorten
the post-stream tail.
"""

import numpy as np

B, N, C = 256, 512, 256
NCORES = 8
BL = B // NCORES      # 32 batches per core
NK = N // 128         # 4 n-chunks of 128
# group sizes for the softmax/t pipeline; smaller tail groups
GROUPS = [8, 8, 8, 4, 4]
# batches per group routed via Pool (V-path); the rest use DVE STT (S-path)
NPOOL = [3, 3, 2, 2, 2]

_CACHE = {}


def _build():
    from contextlib import ExitStack

    import concourse.bass as bass
    import concourse.mybir as mybir
    import concourse.tile as tile
    from concourse import bacc
    from concourse.masks import make_identity

    dt = mybir.dt
    AF = mybir.ActivationFunctionType
    OP = mybir.AluOpType
    f32, f16 = dt.float32, dt.float16

    nc = bacc.Bacc("TRN2", target_bir_lowering=False)
    pt_d = nc.dram_tensor("pt", [128, BL, 2, N], f16, kind="ExternalInput")
    qkt_d = nc.dram_tensor("qkt", [128, 2, BL], f16, kind="ExternalInput")
    tt_d = nc.dram_tensor("tt", [128, 2, BL], f32, kind="ExternalOutput")
    es_d = nc.dram_tensor("es", [BL, 1], f32, kind="ExternalOutput")

    g_base = [sum(GROUPS[:i]) for i in range(len(GROUPS))]

    with tile.TileContext(nc) as tc, ExitStack() as ctx:
        consts = ctx.enter_context(tc.tile_pool(name="consts", bufs=1))
        big = ctx.enter_context(tc.tile_pool(name="big", bufs=1))
        sgrp = ctx.enter_context(tc.tile_pool(name="sgrp", bufs=3))
        scrp = ctx.enter_context(tc.tile_pool(name="scrp", bufs=3))
        vprp = ctx.enter_context(tc.tile_pool(name="vprp", bufs=3))
        psS = ctx.enter_context(tc.tile_pool(name="psS", bufs=1, space="PSUM"))
        psN = ctx.enter_context(tc.tile_pool(name="psN", bufs=2, space="PSUM"))
        psB = ctx.enter_context(tc.tile_pool(name="psB", bufs=4, space="PSUM"))

        ident16 = consts.tile([128, 128], f16)
        make_identity(nc, ident16)
        ident32 = consts.tile([128, 128], f32)
        make_identity(nc, ident32)

        qkt_sb = consts.tile([128, 2, BL], f16)
        nc.sync.dma_start(out=qkt_sb, in_=qkt_d[:, :, :])

        pt_all = big.tile([128, BL, 2, N], f16)
        attnT = consts.tile([128, NK, BL], f16)
        tt_sb = consts.tile([128, 2, BL], f32)

        def load_part(b0, nb):
            nc.sync.dma_start(
                out=pt_all[:, b0 : b0 + nb], in_=pt_d[:, b0 : b0 + nb]
            )

        # scores_cols[:, k, b] = sum_c pt[c, b, :, k*128:(k+1)*128]^T qk[b]
        scores_cols = psS.tile([128, NK, BL], f32)
        # attn-column transpose landing slots (ping-pong per group)
        atT_all = psS.tile([128, 2, NK, 8], f16)

        def scores(b):
            for k in range(NK):
                for ch in range(2):
                    nc.tensor.matmul(
                        out=scores_cols[:, k, b : b + 1],
                        lhsT=pt_all[:, b, ch, k * 128 : (k + 1) * 128],
                        rhs=qkt_sb[:, ch, b : b + 1],
                        start=(ch == 0),
                        stop=(ch == 1),
                    )

        grp_state = {}

        def phase_g1(g):
            # scores PSUM -> SBUF (PE transpose input must be SBUF)
            gb, G = g_base[g], GROUPS[g]
            sc_sb = sgrp.tile([128, NK, 8], f32, tag="sc_sb")
            nc.scalar.copy(
                out=sc_sb[:, :, :G], in_=scores_cols[:, :, gb : gb + G]
            )
            grp_state[g] = sc_sb

        def phase_g2(g):
            # columns -> natural rows [G, 512] in PSUM
            G = GROUPS[g]
            sc_sb = grp_state[g]
            sc_ps = psN.tile([8, NK, 128], f32, tag="scnat")
            for k in range(NK):
                nc.tensor.transpose(
                    out=sc_ps[:G, k, :], in_=sc_sb[:, k, :G], identity=ident32
                )
            grp_state[g] = sc_ps

        def phase_g3(g):
            # row max (safe softmax shift)
            G = GROUPS[g]
            sc_ps = grp_state[g]
            smax = sgrp.tile([8, 1], f32, tag="smax")
            nc.vector.tensor_reduce(
                out=smax[:G],
                in_=sc_ps[:G],
                axis=mybir.AxisListType.XY,
                op=OP.max,
            )
            negmax = sgrp.tile([8, 1], f32, tag="negmax")
            nc.vector.tensor_scalar_mul(negmax[:G], smax[:G], -1.0)
            grp_state[g] = (sc_ps, negmax)

        def phase_g4(g):
            # unnormalized attn = exp(s - max) in fp16 + esum for the host
            gb, G = g_base[g], GROUPS[g]
            sc_ps, negmax = grp_state[g]
            attn_nat = sgrp.tile([8, NK, 128], f16, tag="attn_nat")
            esum = sgrp.tile([8, 1], f32, tag="esum")
            nc.scalar.activation(
                out=attn_nat[:G],
                in_=sc_ps[:G],
                func=AF.Exp,
                bias=negmax[:G, 0:1],
                scale=1.0,
                accum_out=esum[:G],
            )
            # issue on the ACT queue right after the producer so the SP
            # load stream never waits on the softmax chain
            nc.scalar.dma_start(out=es_d[gb : gb + G], in_=esum[:G])
            grp_state[g] = attn_nat

        def phase_g5(g):
            # attn rows -> columns [128, NK, G] (bcast-transpose input)
            gb, G = g_base[g], GROUPS[g]
            attn_nat = grp_state.pop(g)
            atT_ps = atT_all[:, g % 2]
            for k in range(NK):
                nc.tensor.transpose(
                    out=atT_ps[:, k, :G],
                    in_=attn_nat[:G, k, :],
                    identity=ident16[:G, :G],
                )
            nc.scalar.copy(
                out=attnT[:, :, gb : gb + G], in_=atT_ps[:, :, :G]
            )

        def phase_t(g, j):
            # t[:, ch, b] = sum_n pt[:, b, ch, n] * attn[b, n]
            # handles the batch PAIR (j, j+1); one PSUM bank per pair
            gb, G = g_base[g], GROUPS[g]
            ab_ps = psB.tile([128, 2, NK, 128], f16, tag="ab")
            for h in range(2):
                b = gb + j + h
                for k in range(NK):
                    nc.tensor.transpose(
                        out=ab_ps[:, h, k, :],
                        in_=attnT[:, k, b : b + 1].to_broadcast([128, 128]),
                        identity=ident16,
                    )
            for h in range(2):
                b = gb + j + h
                ab_flat = ab_ps[:, h].rearrange("p k f -> p (k f)")
                if j + h >= G - NPOOL[g]:
                    # V-path: Pool muls (needs SBUF attn), ACT accumulates
                    ab_sb = vprp.tile([128, N], f16, tag="ab_sb")
                    nc.scalar.copy(out=ab_sb, in_=ab_flat)
                    prod = vprp.tile([128, 2, N], f16, tag="prod")
                    for ch in range(2):
                        nc.gpsimd.tensor_mul(
                            out=prod[:, ch, :],
                            in0=pt_all[:, b, ch, :],
                            in1=ab_sb,
                        )
                    scr = vprp.tile([128, N], f16, tag="vscr")
                    for ch in range(2):
                        nc.scalar.activation(
                            out=scr,
                            in_=prod[:, ch, :],
                            func=AF.Identity,
                            scale=1.0,
                            accum_out=tt_sb[:, ch, b : b + 1],
                        )
                else:
                    # S-path: fused mul+accum on DVE, attn straight from PSUM
                    scr = scrp.tile([128, N], f16, tag="sscr")
                    for ch in range(2):
                        nc.vector.scalar_tensor_tensor(
                            out=scr,
                            in0=pt_all[:, b, ch, :],
                            scalar=1.0,
                            in1=ab_flat,
                            op0=OP.mult,
                            op1=OP.mult,
                            accum_out=tt_sb[:, ch, b : b + 1],
                        )

        # ---- schedule ----
        # stage list: (fn(g), ready offset in completed batches)
        NG = len(GROUPS)
        g_end = [g_base[i] + GROUPS[i] for i in range(NG)]
        stages = [phase_g1, phase_g2, phase_g3, phase_g4, phase_g5]
        nstage = [0] * (len(stages) + 1)
        tcount = [0] * NG  # per-group emitted t-batches

        def run_stages(done):
            # emit group stage s when the group's batches are done loading
            # and the previous stage for that group has been emitted;
            # stages are staggered so each cross-engine hop has slack
            for si, fn in enumerate(stages):
                lim = nstage[si - 1] if si else NG
                while nstage[si] < lim and done >= min(
                    g_end[nstage[si]] + 2 + si, BL
                ):
                    fn(nstage[si])
                    nstage[si] += 1
            # t-work: emit up to two PAIRS per call so the t-stream keeps
            # pace with the loads; skewed well behind the group chain so
            # the bcast transposes never head-of-line-block the PE queue
            emitted = 0
            for g in range(nstage[len(stages) - 1]):
                while (
                    emitted < 2
                    and tcount[g] < GROUPS[g]
                    and done >= min(g_end[g] + 7, BL - 2)
                ):
                    phase_t(g, tcount[g])
                    tcount[g] += 2
                    emitted += 1

        for b0 in range(0, BL, 2):
            load_part(b0, 2)
            scores(b0)
            scores(b0 + 1)
            run_stages(b0 + 2)

        # drain: finish all stages and t-work in dependency order
        for si, fn in enumerate(stages):
            while nstage[si] < NG:
                fn(nstage[si])
                nstage[si] += 1
        for g in range(NG):
            while tcount[g] < GROUPS[g]:
                phase_t(g, tcount[g])
                tcount[g] += 2

        nc.sync.dma_start(out=tt_d[:, :, :], in_=tt_sb)

    nc.compile()
    return nc


LAST_RESULT = None


def kernel(P, id, Wq, bq, Wk, bk, Wv, bv):
    global LAST_RESULT
    from concourse.bass_utils import run_bass_kernel_spmd

    P = np.asarray(P, dtype=np.float32)
    idv = np.asarray(id).astype(np.int64)
    Wq = np.asarray(Wq, dtype=np.float32)
    Wk = np.asarray(Wk, dtype=np.float32)
    Wv = np.asarray(Wv, dtype=np.float32)
    bq = np.asarray(bq, dtype=np.float32)
    bv = np.asarray(bv, dtype=np.float32)

    if "nc" not in _CACHE:
        _CACHE["nc"] = _build()
    nc = _CACHE["nc"]

    # host-side projection folding: qk = Wk^T (Wq p_sel + bq)
    p_sel = P[np.arange(B), idv]                      # [B, C]
    qk = (p_sel @ Wq.T + bq) @ Wk                     # [B, C]

    in_maps = []
    for c in range(NCORES):
        sl = slice(c * BL, (c + 1) * BL)
        # pt[cp, b, ch, n] = P[b, n, ch*128+cp]
        pc = P[sl].reshape(BL, N, 2, 128).transpose(3, 0, 2, 1)
        qc = qk[sl].reshape(BL, 2, 128).transpose(2, 1, 0)
        in_maps.append(
            {
                "pt": np.ascontiguousarray(pc.astype(np.float16)),
                "qkt": np.ascontiguousarray(qc.astype(np.float16)),
            }
        )

    res = run_bass_kernel_spmd(nc, in_maps, core_ids=list(range(NCORES)))
    LAST_RESULT = res

    outs = []
    for c in range(NCORES):
        tt = res.results[c]["tt"]                     # [128, 2, BL] f32
        es = res.results[c]["es"]                     # [BL, 1] f32
        t_un = tt.transpose(2, 1, 0).reshape(BL, C)   # [BL, C]
        outs.append((t_un / es) @ Wv.T + bv)
    return np.concatenate(outs, axis=0).astype(np.float32)


# revision 20
# speedup vs baseline: 1.1316x; 1.0317x over previous
"""Trainium2 Bass kernel for nn_Attention_20933670601301.

Math (per batch b, P[b] in [n, C], n=512, C=256):
    qk    = Wk^T (Wq P[b, id_b] + bq)            (folded on host)
    scores= P[b] @ qk  (+ const that cancels in softmax)
    attn  = softmax(scores)
    t     = P[b]^T attn        -> out = Wv t + bv (applied on host)

Device work is the O(n)-scaling part only: scores, softmax, t.
P is uploaded once per core as fp16 in C-MAJOR layout pt[cp, b, ch, n]
(c = ch*128 + cp), 8 MiB/core -> ~23.5 us DMA floor at 360 GB/s.

Per core (32 batches):
  scores: PE matmuls with P chunks as the STATIONARY operand
          (lhsT = pt[:, b, ch, k-chunk], rhs = qk column, F=1), accumulated
          over ch into one PSUM tile scores_cols [128, 4, 32].
  softmax (groups of batches): PE-transpose score columns to rows [G, 512],
          DVE max, ACT fused exp(+bias)+sum -> UNNORMALIZED attn fp16 and
          esum (host divides by esum), PE-transpose attn back to columns.
  t:      per batch, broadcast attn across partitions (PE transpose of a
          free-broadcast column -> fp16 PSUM [128, 512]), then the free-dim
          weighted reduction t[c] = sum_n pt[c, n]*attn[n] as either
            S-path: DVE scalar_tensor_tensor (fused mul+accum, PSUM in1)
            V-path: ACT copy attn->SBUF, Pool mul, ACT Identity+accum
          split so DVE/Pool/ACT all stay under the DMA roofline.
Emission is software-pipelined across groups (stage machinery) to avoid
in-order queue head-of-line stalls; later groups are smaller to shorten
the post-stream tail.
"""

import numpy as np

B, N, C = 256, 512, 256
NCORES = 8
BL = B // NCORES      # 32 batches per core
NK = N // 128         # 4 n-chunks of 128
# group sizes for the softmax/t pipeline; small head group for an early
# pipeline start, small tail groups to shrink the post-stream tail
GROUPS = [4, 8, 8, 4, 4, 2, 2]
# batches per group routed via Pool (V-path); the rest use DVE STT (S-path)
NPOOL = [1, 3, 3, 1, 1, 1, 1]

_CACHE = {}


def _build():
    from contextlib import ExitStack

    import concourse.bass as bass
    import concourse.mybir as mybir
    import concourse.tile as tile
    from concourse import bacc
    from concourse.masks import make_identity

    dt = mybir.dt
    AF = mybir.ActivationFunctionType
    OP = mybir.AluOpType
    f32, f16 = dt.float32, dt.float16

    nc = bacc.Bacc("TRN2", target_bir_lowering=False)
    pt_d = nc.dram_tensor("pt", [128, BL, 2, N], f16, kind="ExternalInput")
    qkt_d = nc.dram_tensor("qkt", [128, 2, BL], f16, kind="ExternalInput")
    tt_d = nc.dram_tensor("tt", [128, 2, BL], f32, kind="ExternalOutput")
    es_d = nc.dram_tensor("es", [BL, 1], f32, kind="ExternalOutput")

    g_base = [sum(GROUPS[:i]) for i in range(len(GROUPS))]

    with tile.TileContext(nc) as tc, ExitStack() as ctx:
        consts = ctx.enter_context(tc.tile_pool(name="consts", bufs=1))
        big = ctx.enter_context(tc.tile_pool(name="big", bufs=1))
        sgrp = ctx.enter_context(tc.tile_pool(name="sgrp", bufs=3))
        scrp = ctx.enter_context(tc.tile_pool(name="scrp", bufs=3))
        vprp = ctx.enter_context(tc.tile_pool(name="vprp", bufs=3))
        psS = ctx.enter_context(tc.tile_pool(name="psS", bufs=1, space="PSUM"))
        psN = ctx.enter_context(tc.tile_pool(name="psN", bufs=2, space="PSUM"))
        psB = ctx.enter_context(tc.tile_pool(name="psB", bufs=4, space="PSUM"))

        ident16 = consts.tile([128, 128], f16)
        make_identity(nc, ident16)
        ident32 = consts.tile([128, 128], f32)
        make_identity(nc, ident32)

        qkt_sb = consts.tile([128, 2, BL], f16)
        nc.sync.dma_start(out=qkt_sb, in_=qkt_d[:, :, :])

        pt_all = big.tile([128, BL, 2, N], f16)
        attnT = consts.tile([128, NK, BL], f16)
        tt_sb = consts.tile([128, 2, BL], f32)

        def load_part(b0, nb):
            nc.sync.dma_start(
                out=pt_all[:, b0 : b0 + nb], in_=pt_d[:, b0 : b0 + nb]
            )

        # scores_cols[:, k, b] = sum_c pt[c, b, :, k*128:(k+1)*128]^T qk[b]
        scores_cols = psS.tile([128, NK, BL], f32)
        # attn-column transpose landing slots (ping-pong per group)
        atT_all = psS.tile([128, 2, NK, 8], f16)

        def scores(b):
            for k in range(NK):
                for ch in range(2):
                    nc.tensor.matmul(
                        out=scores_cols[:, k, b : b + 1],
                        lhsT=pt_all[:, b, ch, k * 128 : (k + 1) * 128],
                        rhs=qkt_sb[:, ch, b : b + 1],
                        start=(ch == 0),
                        stop=(ch == 1),
                    )

        grp_state = {}

        def phase_g1(g):
            # scores PSUM -> SBUF (PE transpose input must be SBUF)
            gb, G = g_base[g], GROUPS[g]
            sc_sb = sgrp.tile([128, NK, 8], f32, tag="sc_sb")
            nc.scalar.copy(
                out=sc_sb[:, :, :G], in_=scores_cols[:, :, gb : gb + G]
            )
            grp_state[g] = sc_sb

        def phase_g2(g):
            # columns -> natural rows [G, 512] in PSUM
            G = GROUPS[g]
            sc_sb = grp_state[g]
            sc_ps = psN.tile([8, NK, 128], f32, tag="scnat")
            for k in range(NK):
                nc.tensor.transpose(
                    out=sc_ps[:G, k, :], in_=sc_sb[:, k, :G], identity=ident32
                )
            grp_state[g] = (sc_sb, sc_ps)

        def phase_g3(g):
            # safe-softmax shift: fused row-max + negate
            G = GROUPS[g]
            _sc_sb, sc_ps = grp_state[g]
            negmax = sgrp.tile([8, 1], f32, tag="negmax")
            nc.vector.tensor_reduce(
                out=negmax[:G],
                in_=sc_ps[:G],
                axis=mybir.AxisListType.XY,
                op=OP.max,
                negate=True,
            )
            grp_state[g] = (sc_ps, negmax)

        def phase_g4(g):
            # unnormalized attn = exp(s - max) in fp16 + esum for the host
            gb, G = g_base[g], GROUPS[g]
            sc_ps, negmax = grp_state[g]
            attn_nat = sgrp.tile([8, NK, 128], f16, tag="attn_nat")
            esum = sgrp.tile([8, 1], f32, tag="esum")
            nc.scalar.activation(
                out=attn_nat[:G],
                in_=sc_ps[:G],
                func=AF.Exp,
                bias=negmax[:G, 0:1],
                scale=1.0,
                accum_out=esum[:G],
            )
            # issue on the ACT queue right after the producer so the SP
            # load stream never waits on the softmax chain
            nc.scalar.dma_start(out=es_d[gb : gb + G], in_=esum[:G])
            grp_state[g] = attn_nat

        def phase_g5(g):
            # attn rows -> columns [128, NK, G] (bcast-transpose input)
            gb, G = g_base[g], GROUPS[g]
            attn_nat = grp_state.pop(g)
            atT_ps = atT_all[:, g % 2]
            for k in range(NK):
                nc.tensor.transpose(
                    out=atT_ps[:, k, :G],
                    in_=attn_nat[:G, k, :],
                    identity=ident16[:G, :G],
                )
            nc.scalar.copy(
                out=attnT[:, :, gb : gb + G], in_=atT_ps[:, :, :G]
            )

        def phase_t(g, j):
            # t[:, ch, b] = sum_n pt[:, b, ch, n] * attn[b, n]
            # handles the batch PAIR (j, j+1); one PSUM bank per pair
            gb, G = g_base[g], GROUPS[g]
            ab_ps = psB.tile([128, 2, NK, 128], f16, tag="ab")
            for h in range(2):
                b = gb + j + h
                for k in range(NK):
                    nc.tensor.transpose(
                        out=ab_ps[:, h, k, :],
                        in_=attnT[:, k, b : b + 1].to_broadcast([128, 128]),
                        identity=ident16,
                    )
            for h in range(2):
                b = gb + j + h
                ab_flat = ab_ps[:, h].rearrange("p k f -> p (k f)")
                if j + h >= G - NPOOL[g]:
                    # V-path: Pool muls (needs SBUF attn), ACT accumulates
                    ab_sb = vprp.tile([128, N], f16, tag="ab_sb")
                    nc.scalar.copy(out=ab_sb, in_=ab_flat)
                    prod = vprp.tile([128, 2, N], f16, tag="prod")
                    for ch in range(2):
                        nc.gpsimd.tensor_mul(
                            out=prod[:, ch, :],
                            in0=pt_all[:, b, ch, :],
                            in1=ab_sb,
                        )
                    scr = vprp.tile([128, N], f16, tag="vscr")
                    for ch in range(2):
                        nc.scalar.activation(
                            out=scr,
                            in_=prod[:, ch, :],
                            func=AF.Identity,
                            scale=1.0,
                            accum_out=tt_sb[:, ch, b : b + 1],
                        )
                else:
                    # S-path: fused mul+accum on DVE, attn straight from PSUM
                    scr = scrp.tile([128, N], f16, tag="sscr")
                    for ch in range(2):
                        nc.vector.scalar_tensor_tensor(
                            out=scr,
                            in0=pt_all[:, b, ch, :],
                            scalar=1.0,
                            in1=ab_flat,
                            op0=OP.mult,
                            op1=OP.mult,
                            accum_out=tt_sb[:, ch, b : b + 1],
                        )

        # ---- schedule ----
        # stage list: (fn(g), ready offset in completed batches)
        NG = len(GROUPS)
        g_end = [g_base[i] + GROUPS[i] for i in range(NG)]
        stages = [phase_g1, phase_g2, phase_g3, phase_g4, phase_g5]
        nstage = [0] * (len(stages) + 1)
        tcount = [0] * NG  # per-group emitted t-batches

        def run_stages(done):
            # emit group stage s when the group's batches are done loading
            # and the previous stage for that group has been emitted;
            # stages are staggered so each cross-engine hop has slack
            for si, fn in enumerate(stages):
                lim = nstage[si - 1] if si else NG
                while nstage[si] < lim and done >= min(
                    g_end[nstage[si]] + 2 + si, BL
                ):
                    fn(nstage[si])
                    nstage[si] += 1
            # t-work: emit up to three PAIRS per call so the t-stream keeps
            # pace with the loads; skewed well behind the group chain so
            # the bcast transposes never head-of-line-block the PE queue
            emitted = 0
            for g in range(nstage[len(stages) - 1]):
                while (
                    emitted < 3
                    and tcount[g] < GROUPS[g]
                    and done >= min(g_end[g] + 7, BL)
                ):
                    phase_t(g, tcount[g])
                    tcount[g] += 2
                    emitted += 1

        for b0 in range(0, BL, 2):
            load_part(b0, 2)
            scores(b0)
            scores(b0 + 1)
            run_stages(b0 + 2)

        # drain: finish all stages and t-work in dependency order
        for si, fn in enumerate(stages):
            while nstage[si] < NG:
                fn(nstage[si])
                nstage[si] += 1
        for g in range(NG):
            while tcount[g] < GROUPS[g]:
                phase_t(g, tcount[g])
                tcount[g] += 2

        nc.sync.dma_start(out=tt_d[:, :, :], in_=tt_sb)

    nc.compile()
    return nc


LAST_RESULT = None


def kernel(P, id, Wq, bq, Wk, bk, Wv, bv):
    global LAST_RESULT
    from concourse.bass_utils import run_bass_kernel_spmd

    P = np.asarray(P, dtype=np.float32)
    idv = np.asarray(id).astype(np.int64)
    Wq = np.asarray(Wq, dtype=np.float32)
    Wk = np.asarray(Wk, dtype=np.float32)
    Wv = np.asarray(Wv, dtype=np.float32)
    bq = np.asarray(bq, dtype=np.float32)
    bv = np.asarray(bv, dtype=np.float32)

    if "nc" not in _CACHE:
        _CACHE["nc"] = _build()
    nc = _CACHE["nc"]

    # host-side projection folding: qk = Wk^T (Wq p_sel + bq)
    p_sel = P[np.arange(B), idv]                      # [B, C]
    qk = (p_sel @ Wq.T + bq) @ Wk                     # [B, C]

    in_maps = []
    for c in range(NCORES):
        sl = slice(c * BL, (c + 1) * BL)
        # pt[cp, b, ch, n] = P[b, n, ch*128+cp]
        pc = P[sl].reshape(BL, N, 2, 128).transpose(3, 0, 2, 1)
        qc = qk[sl].reshape(BL, 2, 128).transpose(2, 1, 0)
        in_maps.append(
            {
                "pt": np.ascontiguousarray(pc.astype(np.float16)),
                "qkt": np.ascontiguousarray(qc.astype(np.float16)),
            }
        )

    res = run_bass_kernel_spmd(nc, in_maps, core_ids=list(range(NCORES)))
    LAST_RESULT = res

    outs = []
    for c in range(NCORES):
        tt = res.results[c]["tt"]                     # [128, 2, BL] f32
        es = res.results[c]["es"]                     # [BL, 1] f32
        t_un = tt.transpose(2, 1, 0).reshape(BL, C)   # [BL, C]
        outs.append((t_un / es) @ Wv.T + bv)
    return np.concatenate(outs, axis=0).astype(np.float32)


# revision 23
# speedup vs baseline: 1.1953x; 1.0563x over previous
"""Trainium2 Bass kernel for nn_Attention_20933670601301.

Math (per batch b, P[b] in [n, C], n=512, C=256):
    qk    = Wk^T (Wq P[b, id_b] + bq)            (folded on host)
    scores= P[b] @ qk  (+ const that cancels in softmax)
    attn  = softmax(scores)
    t     = P[b]^T attn        -> out = Wv t + bv (applied on host)

Device work is the O(n)-scaling part only: scores, softmax, t.
P is uploaded once per core as fp16 in C-MAJOR layout pt[cp, b, ch, n]
(c = ch*128 + cp), 8 MiB/core -> ~23.5 us DMA floor at 360 GB/s.

Per core (32 batches):
  scores: PE matmuls with P chunks as the STATIONARY operand
          (lhsT = pt[:, b, ch, k-chunk], rhs = qk column, F=1), accumulated
          over ch into one PSUM tile scores_cols [128, 4, 32].
  softmax (groups of batches): PE-transpose score columns to rows [G, 512],
          DVE max, ACT fused exp(+bias)+sum -> UNNORMALIZED attn fp16 and
          esum (host divides by esum), PE-transpose attn back to columns.
  t:      per batch, broadcast attn across partitions (PE transpose of a
          free-broadcast column -> fp16 PSUM [128, 512]), then the free-dim
          weighted reduction t[c] = sum_n pt[c, n]*attn[n] as either
            S-path: DVE scalar_tensor_tensor (fused mul+accum, PSUM in1)
            V-path: ACT copy attn->SBUF, Pool mul, ACT Identity+accum
          split so DVE/Pool/ACT all stay under the DMA roofline.
Emission is software-pipelined across groups (stage machinery) to avoid
in-order queue head-of-line stalls; later groups are smaller to shorten
the post-stream tail.
"""

import numpy as np

B, N, C = 256, 512, 256
NCORES = 8
BL = B // NCORES      # 32 batches per core
NK = N // 128         # 4 n-chunks of 128
# group sizes for the softmax/t pipeline; small head group for an early
# pipeline start, small tail groups to shrink the post-stream tail
GROUPS = [4, 8, 8, 4, 4, 2, 2]
# batches per group routed via Pool (V-path); the rest use DVE STT (S-path)
NPOOL = [1, 3, 3, 1, 1, 1, 1]

_CACHE = {}


def _build():
    from contextlib import ExitStack

    import concourse.bass as bass
    import concourse.mybir as mybir
    import concourse.tile as tile
    from concourse import bacc
    from concourse.masks import make_identity

    dt = mybir.dt
    AF = mybir.ActivationFunctionType
    OP = mybir.AluOpType
    f32, f16 = dt.float32, dt.float16

    nc = bacc.Bacc("TRN2", target_bir_lowering=False)
    pt_d = nc.dram_tensor("pt", [128, BL, 2, N], f16, kind="ExternalInput")
    qkt_d = nc.dram_tensor("qkt", [128, 2, BL], f16, kind="ExternalInput")
    tt_d = nc.dram_tensor("tt", [128, 2, BL], f32, kind="ExternalOutput")
    es_d = nc.dram_tensor("es", [BL, 1], f32, kind="ExternalOutput")

    g_base = [sum(GROUPS[:i]) for i in range(len(GROUPS))]

    with tile.TileContext(nc) as tc, ExitStack() as ctx:
        consts = ctx.enter_context(tc.tile_pool(name="consts", bufs=1))
        big = ctx.enter_context(tc.tile_pool(name="big", bufs=1))
        sgrp = ctx.enter_context(tc.tile_pool(name="sgrp", bufs=3))
        scrp = ctx.enter_context(tc.tile_pool(name="scrp", bufs=3))
        vprp = ctx.enter_context(tc.tile_pool(name="vprp", bufs=3))
        psS = ctx.enter_context(tc.tile_pool(name="psS", bufs=1, space="PSUM"))
        psN = ctx.enter_context(tc.tile_pool(name="psN", bufs=2, space="PSUM"))
        psB = ctx.enter_context(tc.tile_pool(name="psB", bufs=4, space="PSUM"))

        ident16 = consts.tile([128, 128], f16)
        make_identity(nc, ident16)
        ident32 = consts.tile([128, 128], f32)
        make_identity(nc, ident32)

        qkt_sb = consts.tile([128, 2, BL], f16)
        nc.sync.dma_start(out=qkt_sb, in_=qkt_d[:, :, :])

        pt_all = big.tile([128, BL, 2, N], f16)
        attnT = consts.tile([128, NK, BL], f16)
        tt_sb = consts.tile([128, 2, BL], f32)
        esum_all = consts.tile([8, len(GROUPS)], f32)

        def load_part(b0, nb):
            nc.sync.dma_start(
                out=pt_all[:, b0 : b0 + nb], in_=pt_d[:, b0 : b0 + nb]
            )

        # scores_cols[:, k, b] = sum_c pt[c, b, :, k*128:(k+1)*128]^T qk[b]
        scores_cols = psS.tile([128, NK, BL], f32)
        # attn-column transpose landing slots (ping-pong per group)
        atT_all = psS.tile([128, 2, NK, 8], f16)

        def scores(b):
            for k in range(NK):
                for ch in range(2):
                    nc.tensor.matmul(
                        out=scores_cols[:, k, b : b + 1],
                        lhsT=pt_all[:, b, ch, k * 128 : (k + 1) * 128],
                        rhs=qkt_sb[:, ch, b : b + 1],
                        start=(ch == 0),
                        stop=(ch == 1),
                    )

        grp_state = {}

        def phase_g1(g):
            # scores PSUM -> SBUF (PE transpose input must be SBUF)
            gb, G = g_base[g], GROUPS[g]
            sc_sb = sgrp.tile([128, NK, 8], f32, tag="sc_sb")
            nc.scalar.copy(
                out=sc_sb[:, :, :G], in_=scores_cols[:, :, gb : gb + G]
            )
            grp_state[g] = sc_sb

        def phase_g2(g):
            # columns -> natural rows [G, 512] in PSUM
            G = GROUPS[g]
            sc_sb = grp_state[g]
            sc_ps = psN.tile([8, NK, 128], f32, tag="scnat")
            for k in range(NK):
                nc.tensor.transpose(
                    out=sc_ps[:G, k, :], in_=sc_sb[:, k, :G], identity=ident32
                )
            grp_state[g] = (sc_sb, sc_ps)

        def phase_g3(g):
            # safe-softmax shift: fused row-max + negate
            G = GROUPS[g]
            _sc_sb, sc_ps = grp_state[g]
            negmax = sgrp.tile([8, 1], f32, tag="negmax")
            nc.vector.tensor_reduce(
                out=negmax[:G],
                in_=sc_ps[:G],
                axis=mybir.AxisListType.XY,
                op=OP.max,
                negate=True,
            )
            grp_state[g] = (sc_ps, negmax)

        def phase_g4(g):
            # unnormalized attn = exp(s - max) in fp16 + esum for the host
            gb, G = g_base[g], GROUPS[g]
            sc_ps, negmax = grp_state[g]
            attn_nat = sgrp.tile([8, NK, 128], f16, tag="attn_nat")
            nc.scalar.activation(
                out=attn_nat[:G],
                in_=sc_ps[:G],
                func=AF.Exp,
                bias=negmax[:G, 0:1],
                scale=1.0,
                accum_out=esum_all[:G, g : g + 1],
            )
            grp_state[g] = attn_nat

        def phase_g5(g):
            # attn rows -> columns [128, NK, G] (bcast-transpose input)
            gb, G = g_base[g], GROUPS[g]
            attn_nat = grp_state.pop(g)
            atT_ps = atT_all[:, g % 2]
            for k in range(NK):
                nc.tensor.transpose(
                    out=atT_ps[:, k, :G],
                    in_=attn_nat[:G, k, :],
                    identity=ident16[:G, :G],
                )
            nc.scalar.copy(
                out=attnT[:, :, gb : gb + G], in_=atT_ps[:, :, :G]
            )

        def phase_t(g, j):
            # t[:, ch, b] = sum_n pt[:, b, ch, n] * attn[b, n]
            # handles the batch PAIR (j, j+1); one PSUM bank per pair
            gb, G = g_base[g], GROUPS[g]
            ab_ps = psB.tile([128, 2, NK, 128], f16, tag="ab")
            for h in range(2):
                b = gb + j + h
                for k in range(NK):
                    nc.tensor.transpose(
                        out=ab_ps[:, h, k, :],
                        in_=attnT[:, k, b : b + 1].to_broadcast([128, 128]),
                        identity=ident16,
                    )
            for h in range(2):
                b = gb + j + h
                ab_flat = ab_ps[:, h].rearrange("p k f -> p (k f)")
                if j + h >= G - NPOOL[g]:
                    # V-path: Pool muls (needs SBUF attn), ACT accumulates
                    ab_sb = vprp.tile([128, N], f16, tag="ab_sb")
                    nc.scalar.copy(out=ab_sb, in_=ab_flat)
                    prod = vprp.tile([128, 2, N], f16, tag="prod")
                    for ch in range(2):
                        nc.gpsimd.tensor_mul(
                            out=prod[:, ch, :],
                            in0=pt_all[:, b, ch, :],
                            in1=ab_sb,
                        )
                    scr = vprp.tile([128, N], f16, tag="vscr")
                    for ch in range(2):
                        nc.scalar.activation(
                            out=scr,
                            in_=prod[:, ch, :],
                            func=AF.Identity,
                            scale=1.0,
                            accum_out=tt_sb[:, ch, b : b + 1],
                        )
                else:
                    # S-path: fused mul+accum on DVE, attn straight from PSUM
                    scr = scrp.tile([128, N], f16, tag="sscr")
                    for ch in range(2):
                        nc.vector.scalar_tensor_tensor(
                            out=scr,
                            in0=pt_all[:, b, ch, :],
                            scalar=1.0,
                            in1=ab_flat,
                            op0=OP.mult,
                            op1=OP.mult,
                            accum_out=tt_sb[:, ch, b : b + 1],
                        )

        # ---- schedule ----
        # stage list: (fn(g), ready offset in completed batches)
        NG = len(GROUPS)
        g_end = [g_base[i] + GROUPS[i] for i in range(NG)]
        stages = [phase_g1, phase_g2, phase_g3, phase_g4, phase_g5]
        nstage = [0] * (len(stages) + 1)
        tcount = [0] * NG  # per-group emitted t-batches

        def run_stages(done):
            # emit group stage s when the group's batches are done loading
            # and the previous stage for that group has been emitted;
            # stages are staggered so each cross-engine hop has slack
            for si, fn in enumerate(stages):
                lim = nstage[si - 1] if si else NG
                while nstage[si] < lim and done >= min(
                    g_end[nstage[si]] + 2 + si, BL
                ):
                    fn(nstage[si])
                    nstage[si] += 1
            # t-work: emit up to three PAIRS per call so the t-stream keeps
            # pace with the loads; skewed well behind the group chain so
            # the bcast transposes never head-of-line-block the PE queue
            emitted = 0
            for g in range(nstage[len(stages) - 1]):
                while (
                    emitted < 3
                    and tcount[g] < GROUPS[g]
                    and done >= min(g_end[g] + 7, BL)
                ):
                    phase_t(g, tcount[g])
                    tcount[g] += 2
                    emitted += 1

        for b0 in range(0, BL, 2):
            load_part(b0, 2)
            scores(b0)
            scores(b0 + 1)
            run_stages(b0 + 2)

        # drain: finish all stages and t-work in dependency order
        for si, fn in enumerate(stages):
            while nstage[si] < NG:
                fn(nstage[si])
                nstage[si] += 1
        for g in range(NG):
            while tcount[g] < GROUPS[g]:
                phase_t(g, tcount[g])
                tcount[g] += 2

        # results out: all deferred to the end so no mid-stream queue ever
        # waits on the softmax/reduce chains
        for g in range(NG):
            nc.sync.dma_start(
                out=es_d[g_base[g] : g_end[g]], in_=esum_all[: GROUPS[g], g : g + 1]
            )
        nc.sync.dma_start(out=tt_d[:, :, :], in_=tt_sb)

    nc.compile()
    return nc


LAST_RESULT = None


def kernel(P, id, Wq, bq, Wk, bk, Wv, bv):
    global LAST_RESULT
    from concourse.bass_utils import run_bass_kernel_spmd

    P = np.asarray(P, dtype=np.float32)
    idv = np.asarray(id).astype(np.int64)
    Wq = np.asarray(Wq, dtype=np.float32)
    Wk = np.asarray(Wk, dtype=np.float32)
    Wv = np.asarray(Wv, dtype=np.float32)
    bq = np.asarray(bq, dtype=np.float32)
    bv = np.asarray(bv, dtype=np.float32)

    if "nc" not in _CACHE:
        _CACHE["nc"] = _build()
    nc = _CACHE["nc"]

    # host-side projection folding: qk = Wk^T (Wq p_sel + bq)
    p_sel = P[np.arange(B), idv]                      # [B, C]
    qk = (p_sel @ Wq.T + bq) @ Wk                     # [B, C]

    in_maps = []
    for c in range(NCORES):
        sl = slice(c * BL, (c + 1) * BL)
        # pt[cp, b, ch, n] = P[b, n, ch*128+cp]
        pc = P[sl].reshape(BL, N, 2, 128).transpose(3, 0, 2, 1)
        qc = qk[sl].reshape(BL, 2, 128).transpose(2, 1, 0)
        in_maps.append(
            {
                "pt": np.ascontiguousarray(pc.astype(np.float16)),
                "qkt": np.ascontiguousarray(qc.astype(np.float16)),
            }
        )

    res = run_bass_kernel_spmd(nc, in_maps, core_ids=list(range(NCORES)))
    LAST_RESULT = res

    outs = []
    for c in range(NCORES):
        tt = res.results[c]["tt"]                     # [128, 2, BL] f32
        es = res.results[c]["es"]                     # [BL, 1] f32
        t_un = tt.transpose(2, 1, 0).reshape(BL, C)   # [BL, C]
        outs.append((t_un / es) @ Wv.T + bv)
    return np.concatenate(outs, axis=0).astype(np.float32)


# revision 37
# speedup vs baseline: 1.5556x; 1.3014x over previous
"""Trainium2 Bass kernel for nn_Attention_20933670601301.

Math (per batch b, P[b] in [n, C], n=512, C=256):
    qk    = Wk^T (Wq P[b, id_b] + bq)            (folded on host)
    scores= P[b] @ qk  (+ const that cancels in softmax)
    attn  = softmax(scores)
    t     = P[b]^T attn        -> out = Wv t + bv (applied on host)

Device work is the O(n)-scaling part only: scores, softmax, t.
P is uploaded once per core as fp16 in C-MAJOR layout pt[cp, b, ch, n]
(c = ch*128 + cp), 8 MiB/core -> ~23.5 us DMA floor at 360 GB/s.

Per core (32 batches):
  scores: PE matmuls with P chunks as the STATIONARY operand
          (lhsT = pt[:, b, ch, k-chunk], rhs = qk column, F=1), accumulated
          over ch into one PSUM tile scores_cols [128, 4, 32].
  softmax (groups of batches): PE-transpose score columns to rows [G, 512],
          DVE max, ACT fused exp(+bias)+sum -> UNNORMALIZED attn fp16 and
          esum (host divides by esum), PE-transpose attn back to columns.
  t:      per batch, broadcast attn across partitions (PE transpose of a
          free-broadcast column -> fp16 PSUM [128, 512]), then the free-dim
          weighted reduction t[c] = sum_n pt[c, n]*attn[n] as either
            S-path: DVE scalar_tensor_tensor (fused mul+accum, PSUM in1)
            V-path: ACT copy attn->SBUF, Pool mul, ACT Identity+accum
          split so DVE/Pool/ACT all stay under the DMA roofline.
Emission is software-pipelined across groups (stage machinery) to avoid
in-order queue head-of-line stalls; later groups are smaller to shorten
the post-stream tail.
"""

import numpy as np

B, N, C = 256, 512, 256
NCORES = 8
BL = B // NCORES      # 32 batches per core
NK = N // 128         # 4 n-chunks of 128
# group sizes for the softmax/t pipeline; small head group for an early
# pipeline start, small tail groups to shrink the post-stream tail
import os as _os
GROUPS = [int(x) for x in _os.environ.get("K_GROUPS", "4,4,8,8,4,4").split(",")]
# batches per group routed via Pool (V-path); the rest use DVE STT (S-path)
NPOOL = [int(x) for x in _os.environ.get("K_NPOOL", "1,1,3,3,2,1").split(",")]
K_G0CHAIN = int(_os.environ.get("K_G0CHAIN", "0"))
K_BULK0 = int(_os.environ.get("K_BULK0", "5"))
K_BULK = int(_os.environ.get("K_BULK", "5"))

_CACHE = {}


def _build():
    from contextlib import ExitStack

    import concourse.bass as bass
    import concourse.mybir as mybir
    import concourse.tile as tile
    from concourse import bacc
    from concourse.masks import make_identity

    dt = mybir.dt
    AF = mybir.ActivationFunctionType
    OP = mybir.AluOpType
    f32, f16 = dt.float32, dt.float16

    nc = bacc.Bacc("TRN2", target_bir_lowering=False)
    pt_d = nc.dram_tensor("pt", [128, BL, 2, N], f16, kind="ExternalInput")
    qkt_d = nc.dram_tensor("qkt", [128, 2, BL], f16, kind="ExternalInput")
    tt_d = nc.dram_tensor("tt", [128, 2, BL], f32, kind="ExternalOutput")
    es_d = nc.dram_tensor("es", [BL, 1], f32, kind="ExternalOutput")

    g_base = [sum(GROUPS[:i]) for i in range(len(GROUPS))]

    with tile.TileContext(nc) as tc, ExitStack() as ctx:
        consts = ctx.enter_context(tc.tile_pool(name="consts", bufs=1))
        big = ctx.enter_context(tc.tile_pool(name="big", bufs=1))
        sgrp = ctx.enter_context(tc.tile_pool(name="sgrp", bufs=3))
        scrp = ctx.enter_context(tc.tile_pool(name="scrp", bufs=3))
        vprp = ctx.enter_context(tc.tile_pool(name="vprp", bufs=4))
        psS = ctx.enter_context(tc.tile_pool(name="psS", bufs=1, space="PSUM"))
        psN = ctx.enter_context(tc.tile_pool(name="psN", bufs=2, space="PSUM"))
        psB = ctx.enter_context(tc.tile_pool(name="psB", bufs=4, space="PSUM"))

        ident16 = consts.tile([128, 128], f16)
        make_identity(nc, ident16)
        ident32 = consts.tile([128, 128], f32)
        make_identity(nc, ident32)

        qkt_sb = consts.tile([128, 2, BL], f16)
        nc.sync.dma_start(out=qkt_sb, in_=qkt_d[:, :, :])

        pt_all = big.tile([128, BL, 2, N], f16)
        attnT = consts.tile([128, NK, BL], f16)
        tt_sb = consts.tile([128, 2, BL], f32)
        esum_all = consts.tile([8, len(GROUPS)], f32)

        def load_part(b0, nb):
            nc.sync.dma_start(
                out=pt_all[:, b0 : b0 + nb], in_=pt_d[:, b0 : b0 + nb]
            )

        # scores_cols[:, k, b] = sum_c pt[c, b, :, k*128:(k+1)*128]^T qk[b]
        scores_cols = psS.tile([128, NK, BL], f32)
        # attn-column transpose landing slots (ping-pong per group)
        atT_all = psS.tile([128, 2, NK, 8], f16)

        def scores(b):
            for k in range(NK):
                for ch in range(2):
                    nc.tensor.matmul(
                        out=scores_cols[:, k, b : b + 1],
                        lhsT=pt_all[:, b, ch, k * 128 : (k + 1) * 128],
                        rhs=qkt_sb[:, ch, b : b + 1],
                        start=(ch == 0),
                        stop=(ch == 1),
                    )

        grp_state = {}

        def phase_g1(g):
            # scores PSUM -> SBUF (PE transpose input must be SBUF)
            gb, G = g_base[g], GROUPS[g]
            sc_sb = sgrp.tile([128, NK, 8], f32, tag="sc_sb")
            nc.scalar.copy(
                out=sc_sb[:, :, :G], in_=scores_cols[:, :, gb : gb + G]
            )
            grp_state[g] = sc_sb

        def phase_g2(g):
            # columns -> natural rows [G, 512] in PSUM
            G = GROUPS[g]
            sc_sb = grp_state[g]
            sc_ps = psN.tile([8, NK, 128], f32, tag="scnat")
            for k in range(NK):
                nc.tensor.transpose(
                    out=sc_ps[:G, k, :], in_=sc_sb[:, k, :G], identity=ident32
                )
            grp_state[g] = (sc_sb, sc_ps)

        def phase_g3(g):
            # safe-softmax shift: fused row-max + negate
            G = GROUPS[g]
            _sc_sb, sc_ps = grp_state[g]
            negmax = sgrp.tile([8, 1], f32, tag="negmax")
            nc.vector.tensor_reduce(
                out=negmax[:G],
                in_=sc_ps[:G],
                axis=mybir.AxisListType.XY,
                op=OP.max,
                negate=True,
            )
            grp_state[g] = (sc_ps, negmax)

        def phase_g4(g):
            # unnormalized attn = exp(s - max) in fp16 + esum for the host
            gb, G = g_base[g], GROUPS[g]
            sc_ps, negmax = grp_state[g]
            attn_nat = sgrp.tile([8, NK, 128], f16, tag="attn_nat")
            nc.scalar.activation(
                out=attn_nat[:G],
                in_=sc_ps[:G],
                func=AF.Exp,
                bias=negmax[:G, 0:1],
                scale=1.0,
                accum_out=esum_all[:G, g : g + 1],
            )
            grp_state[g] = attn_nat

        def phase_g5(g):
            # attn rows -> columns [128, NK, G] (bcast-transpose input)
            gb, G = g_base[g], GROUPS[g]
            attn_nat = grp_state.pop(g)
            atT_ps = atT_all[:, g % 2]
            for k in range(NK):
                nc.tensor.transpose(
                    out=atT_ps[:, k, :G],
                    in_=attn_nat[:G, k, :],
                    identity=ident16[:G, :G],
                )
            nc.scalar.copy(
                out=attnT[:, :, gb : gb + G], in_=atT_ps[:, :, :G]
            )

        vpost = []  # deferred V-batch ACT accumulates (break ACT<->Pool ping-pong)

        def flush_vpost(keep):
            while len(vpost) > keep:
                b, prod = vpost.pop(0)
                scr = vprp.tile([128, N], f16, tag="vscr")
                for ch in range(2):
                    nc.scalar.activation(
                        out=scr,
                        in_=prod[:, ch, :],
                        func=AF.Identity,
                        scale=1.0,
                        accum_out=tt_sb[:, ch, b : b + 1],
                    )

        def phase_t(g, j):
            # t[:, ch, b] = sum_n pt[:, b, ch, n] * attn[b, n]
            # handles the batch PAIR (j, j+1); one PSUM bank per pair
            gb, G = g_base[g], GROUPS[g]
            ab_ps = psB.tile([128, 2, NK, 128], f16, tag="ab")
            for h in range(2):
                b = gb + j + h
                for k in range(NK):
                    nc.tensor.transpose(
                        out=ab_ps[:, h, k, :],
                        in_=attnT[:, k, b : b + 1].to_broadcast([128, 128]),
                        identity=ident16,
                    )
            for h in range(2):
                b = gb + j + h
                ab_flat = ab_ps[:, h].rearrange("p k f -> p (k f)")
                if j + h >= G - NPOOL[g]:
                    # V-path: Pool muls (needs SBUF attn); ACT accumulates
                    # later (deferred via vpost)
                    ab_sb = vprp.tile([128, N], f16, tag="ab_sb")
                    nc.scalar.copy(out=ab_sb, in_=ab_flat)
                    prod = vprp.tile([128, 2, N], f16, tag="prod")
                    for ch in range(2):
                        nc.gpsimd.tensor_mul(
                            out=prod[:, ch, :],
                            in0=pt_all[:, b, ch, :],
                            in1=ab_sb,
                        )
                    vpost.append((b, prod))
                else:
                    # S-path: fused mul+accum on DVE, attn straight from PSUM
                    scr = scrp.tile([128, N], f16, tag="sscr")
                    for ch in range(2):
                        nc.vector.scalar_tensor_tensor(
                            out=scr,
                            in0=pt_all[:, b, ch, :],
                            scalar=1.0,
                            in1=ab_flat,
                            op0=OP.mult,
                            op1=OP.mult,
                            accum_out=tt_sb[:, ch, b : b + 1],
                        )

        # ---- schedule ----
        # stage list: (fn(g), ready offset in completed batches)
        NG = len(GROUPS)
        g_end = [g_base[i] + GROUPS[i] for i in range(NG)]
        stages = [phase_g1, phase_g2, phase_g3, phase_g4, phase_g5]
        nstage = [0] * (len(stages) + 1)
        # per-group pair emission order: V pairs (high j) first so the
        # Pool/ACT feed starts right after the group chain
        pair_order = [list(range(GROUPS[g] - 2, -1, -2)) for g in range(NG)]
        tcount = [0] * NG  # per-group emitted t-pairs (index into pair_order)

        def run_stages(done):
            # emit group stage s when the group's batches are done loading
            # and the previous stage for that group has been emitted;
            # stages are staggered so each cross-engine hop has slack
            for si, fn in enumerate(stages):
                lim = nstage[si - 1] if si else NG
                while nstage[si] < lim and done >= min(
                    g_end[nstage[si]] + (K_G0CHAIN if nstage[si] == 0 else 2), BL
                ):
                    fn(nstage[si])
                    nstage[si] += 1
            # t-work: emit up to three PAIRS per call so the t-stream keeps
            # pace with the loads; skewed behind the group chain so the
            # bcast transposes never head-of-line-block the PE queue
            emitted = 0
            for g in range(nstage[len(stages) - 1]):
                gate = g_end[g] + (K_BULK0 if g == 0 else K_BULK)
                while (
                    emitted < 3
                    and tcount[g] < len(pair_order[g])
                    and done >= min(gate, BL)
                ):
                    phase_t(g, pair_order[g][tcount[g]])
                    tcount[g] += 1
                    emitted += 1
            flush_vpost(2)

        for b0 in range(0, BL, 2):
            load_part(b0, 2)
            scores(b0)
            scores(b0 + 1)
            run_stages(b0 + 2)

        # drain: finish all stages and t-work in dependency order
        for si, fn in enumerate(stages):
            while nstage[si] < NG:
                fn(nstage[si])
                nstage[si] += 1
        for g in range(NG):
            while tcount[g] < len(pair_order[g]):
                phase_t(g, pair_order[g][tcount[g]])
                tcount[g] += 1
                flush_vpost(1)
        flush_vpost(0)

        # results out: all deferred to the end so no mid-stream queue ever
        # waits on the softmax/reduce chains
        for g in range(NG):
            nc.sync.dma_start(
                out=es_d[g_base[g] : g_end[g]], in_=esum_all[: GROUPS[g], g : g + 1]
            )
        nc.sync.dma_start(out=tt_d[:, :, :], in_=tt_sb)

    nc.compile()
    return nc


LAST_RESULT = None


def kernel(P, id, Wq, bq, Wk, bk, Wv, bv):
    global LAST_RESULT
    from concourse.bass_utils import run_bass_kernel_spmd

    P = np.asarray(P, dtype=np.float32)
    idv = np.asarray(id).astype(np.int64)
    Wq = np.asarray(Wq, dtype=np.float32)
    Wk = np.asarray(Wk, dtype=np.float32)
    Wv = np.asarray(Wv, dtype=np.float32)
    bq = np.asarray(bq, dtype=np.float32)
    bv = np.asarray(bv, dtype=np.float32)

    if "nc" not in _CACHE:
        _CACHE["nc"] = _build()
    nc = _CACHE["nc"]

    # host-side projection folding: qk = Wk^T (Wq p_sel + bq)
    p_sel = P[np.arange(B), idv]                      # [B, C]
    qk = (p_sel @ Wq.T + bq) @ Wk                     # [B, C]

    in_maps = []
    for c in range(NCORES):
        sl = slice(c * BL, (c + 1) * BL)
        # pt[cp, b, ch, n] = P[b, n, ch*128+cp]
        pc = P[sl].reshape(BL, N, 2, 128).transpose(3, 0, 2, 1)
        qc = qk[sl].reshape(BL, 2, 128).transpose(2, 1, 0)
        in_maps.append(
            {
                "pt": np.ascontiguousarray(pc.astype(np.float16)),
                "qkt": np.ascontiguousarray(qc.astype(np.float16)),
            }
        )

    res = run_bass_kernel_spmd(nc, in_maps, core_ids=list(range(NCORES)))
    LAST_RESULT = res

    outs = []
    for c in range(NCORES):
        tt = res.results[c]["tt"]                     # [128, 2, BL] f32
        es = res.results[c]["es"]                     # [BL, 1] f32
        t_un = tt.transpose(2, 1, 0).reshape(BL, C)   # [BL, C]
        outs.append((t_un / es) @ Wv.T + bv)
    return np.concatenate(outs, axis=0).astype(np.float32)
